# revision 29
# baseline (speedup 1.0000x reference)
"""Cross-attention Trainium2 kernel (8 NeuronCores, SPMD), v4 "streamed".

Sharding: core = 2*b + hh  (b = batch 0..3, hh = head-half 0..1).
Each core computes attention for one batch and 8 of the 16 heads, plus the
partial output projection for its head block; the host sums the two partial
projections per batch.

v4 vs v3 (391us): the v3 structure ran all K/V projections first (~70us
with the ACT engine idle) and only then the ACT-bound attention. v4
streams over source chunks: as soon as chunk 0's K/V/Q are projected,
attention bursts start; each burst (head-pair x t-chunk) processes the
current source chunk's st-tiles and folds the partial attn@V sum into an
SBUF accumulator (fp16). Projections for later chunks ride the pop-item
slots inside the bursts. The last source chunk is processed in the v3
per-pair pipelined style, seeding the PSUM accumulation from the SBUF
accumulators, followed by softmax normalization and the out-projection.
The exp stream on ACT therefore starts at ~30us instead of ~88us.
"""

import os
import sys

import numpy as np


def _ensure_paths():
    for p in ("/opt/trn_rl_repo", "/root/.axon_site/_ro/trn_rl_repo"):
        if os.path.isdir(p) and p not in sys.path:
            sys.path.insert(0, p)


_ensure_paths()

import concourse.bass as bass  # noqa: E402
import concourse.mybir as mybir  # noqa: E402
from concourse import bacc  # noqa: E402
from concourse.bass_utils import run_bass_kernel_spmd  # noqa: E402
from concourse.tile import TileContext  # noqa: E402

B, S, T, D, H = 4, 2048, 2048, 1024, 16
HD = D // H  # 64
HL = H // 2  # heads per core: 8
HDL = HL * HD  # 512 head dims per core
PAIRS = HL // 2  # 4 head pairs (2 heads share a 128-partition tile)
KT = D // 128  # 8 contraction k-tiles for the projections
CH = 4  # t-chunks of 512
CHW = 512
F32 = mybir.dt.float32
F16 = mybir.dt.float16

N_CORES = 8

_PROGS = {}
_LAST_ST_N = None
_last_in_maps = None


def _build_program(st_n):
    sp = st_n * 128  # padded (compacted) source length
    scn = (sp + CHW - 1) // CHW  # source stream chunks of 512
    spad = scn * CHW
    bst0 = (scn - 1) * 4  # first st-tile of phase B (last source chunk)

    nc = bacc.Bacc(None, target_bir_lowering=False, debug=False)

    tgtS = nc.dram_tensor("tgtS", [CH, 128, KT, CHW], F16, kind="ExternalInput")
    srcS = nc.dram_tensor("srcS", [scn, 128, KT, CHW], F16, kind="ExternalInput")
    wqS = nc.dram_tensor("wqS", [128, KT, HDL], F16, kind="ExternalInput")
    wkS = nc.dram_tensor("wkS", [128, KT, HDL], F16, kind="ExternalInput")
    wvS = nc.dram_tensor("wvS", [128, KT, HDL], F16, kind="ExternalInput")
    woW = nc.dram_tensor("woW", [128, PAIRS, D], F16, kind="ExternalInput")
    validS = nc.dram_tensor("validS", [128, st_n, HL], F16, kind="ExternalInput")
    outp = nc.dram_tensor("outp", [T, D], F32, kind="ExternalOutput")

    Exp = mybir.ActivationFunctionType.Exp

    with nc.allow_low_precision("fp16 matmul inputs"), TileContext(nc) as tc:
        with (
            tc.tile_pool(name="w", bufs=1) as w_pool,
            tc.tile_pool(name="kv", bufs=1) as kv_pool,
            tc.tile_pool(name="stream", bufs=2) as stream_pool,
            tc.tile_pool(name="tgt", bufs=3) as tgt_pool,
            tc.tile_pool(name="qc", bufs=4) as qc_pool,
            tc.tile_pool(name="pt", bufs=2) as pt_pool,
            tc.tile_pool(name="on", bufs=2) as on_pool,
            tc.tile_pool(name="osb", bufs=2) as osb_pool,
            tc.tile_pool(name="rcb", bufs=2) as rcb_pool,
            tc.tile_pool(name="avs", bufs=2) as avs_pool,
            tc.tile_pool(name="acc_ps", bufs=2, space="PSUM") as acc_ps,
            tc.tile_pool(name="av_ps", bufs=1, space="PSUM") as av_ps_pool,
            tc.tile_pool(name="sc_ps", bufs=2, space="PSUM") as sc_ps_pool,
        ):
            # Startup DMAs split across the two hardware DGE rings (SP =
            # nc.sync, Act = nc.scalar). The gating order: WK+SRC0 (first
            # matmul), then WQ/TGT0 (Q c0), then WV (V c0), then the rest.
            WK = w_pool.tile([128, KT, HDL], F16, tag="wk")
            nc.sync.dma_start(out=WK, in_=wkS[:, :, :])
            SRC0 = stream_pool.tile([128, KT, CHW], F16, tag="stream", name="SRC")
            nc.scalar.dma_start(out=SRC0, in_=srcS[0, :, :, :])
            WQ = w_pool.tile([128, KT, HDL], F16, tag="wq")
            nc.sync.dma_start(out=WQ, in_=wqS[:, :, :])
            TGT0 = tgt_pool.tile([128, KT, CHW], F16, tag="tgt", name="TGT")
            nc.scalar.dma_start(out=TGT0, in_=tgtS[0, :, :, :])
            WV = w_pool.tile([128, KT, HDL], F16, tag="wv")
            nc.scalar.dma_start(out=WV, in_=wvS[:, :, :])
            src_tiles = {0: SRC0}
            SRC1 = stream_pool.tile([128, KT, CHW], F16, tag="stream", name="SRC")
            nc.sync.dma_start(out=SRC1, in_=srcS[1, :, :, :])
            src_tiles[1] = SRC1
            tgt_tiles = {0: TGT0}
            TGT1 = tgt_pool.tile([128, KT, CHW], F16, tag="tgt", name="TGT")
            nc.scalar.dma_start(out=TGT1, in_=tgtS[1, :, :, :])
            tgt_tiles[1] = TGT1
            WO = w_pool.tile([128, PAIRS, D], F16, tag="wo", name="WO")
            nc.sync.dma_start(out=WO, in_=woW[:, :, :])
            VST = w_pool.tile([128, st_n, HL], F16, tag="vst", name="VST")
            nc.scalar.dma_start(out=VST, in_=validS[:, :, :])
            for tcq in (2, 3):
                TGTn = tgt_pool.tile([128, KT, CHW], F16, tag="tgt", name="TGT")
                nc.scalar.dma_start(out=TGTn, in_=tgtS[tcq, :, :, :])
                tgt_tiles[tcq] = TGTn

            # persistent K/V for the whole attention
            KTt = kv_pool.tile([128, PAIRS, spad], F16)
            VON = kv_pool.tile([128, st_n, HL * (HD + 1)], F16)
            von_heads = VON[:, :, :].rearrange("p s (h e) -> p s h e", e=HD + 1)
            nc.vector.tensor_copy(von_heads[:, :, :, HD], VST[:, :, :])
            # fp16 accumulators for the streamed partial attn@V sums: one
            # [65, 2*CHW] slab per (t-chunk, pair)
            ACC = kv_pool.tile([128, CH * PAIRS, 2 * CHW], F16, name="ACC")

            QT_tiles = {}

            # ---- granule emitters (2 matmuls each, sized to the per-st
            # slack of the ACT-bound burst loop) ----
            def emit_kproj(c, j, part):
                # K^T projection for source chunk c, pair j, k-tiles 2*part..
                if part == 0:
                    emit_kproj.ps[(c, j)] = acc_ps.tile(
                        [128, CHW], F32, tag="acc", name="k_ps"
                    )
                k_ps = emit_kproj.ps[(c, j)]
                SRC = src_tiles[c]
                for k in (2 * part, 2 * part + 1):
                    nc.tensor.matmul(
                        k_ps,
                        lhsT=WK[:, k, j * 128 : (j + 1) * 128],
                        rhs=SRC[:, k, :],
                        start=(k == 0),
                        stop=(k == KT - 1),
                    )
                if part == 3:
                    del emit_kproj.ps[(c, j)]
                    nc.vector.tensor_copy(KTt[:, j, c * CHW : (c + 1) * CHW], k_ps)

            emit_kproj.ps = {}

            def emit_vproj(c, stl, part):
                st = c * 4 + stl
                if st >= st_n:
                    return
                if part == 0:
                    emit_vproj.ps[st] = acc_ps.tile(
                        [128, CHW], F32, tag="acc", name="v_ps"
                    )
                v_ps = emit_vproj.ps[st]
                SRC = src_tiles[c]
                for k in (2 * part, 2 * part + 1):
                    nc.tensor.matmul(
                        v_ps[:, 0:HDL],
                        lhsT=SRC[:, k, stl * 128 : (stl + 1) * 128],
                        rhs=WV[:, k, :],
                        start=(k == 0),
                        stop=(k == KT - 1),
                    )
                if part == 3:
                    del emit_vproj.ps[st]
                    nc.vector.tensor_copy(
                        von_heads[:, st, :, 0:HD],
                        v_ps[:, 0:HDL].rearrange("p (h e) -> p h e", e=HD),
                    )

            emit_vproj.ps = {}

            def emit_qproj(tcq, j, part):
                if part == 0 and j == 0:
                    QT_tiles[tcq] = qc_pool.tile(
                        [128, PAIRS, CHW], F16, name="QTc"
                    )
                if part == 0:
                    emit_qproj.ps[(tcq, j)] = acc_ps.tile(
                        [128, CHW], F32, tag="acc", name="q_ps"
                    )
                q_ps = emit_qproj.ps[(tcq, j)]
                TGT = tgt_tiles[tcq]
                for k in (2 * part, 2 * part + 1):
                    nc.tensor.matmul(
                        q_ps,
                        lhsT=WQ[:, k, j * 128 : (j + 1) * 128],
                        rhs=TGT[:, k, :],
                        start=(k == 0),
                        stop=(k == KT - 1),
                    )
                if part == 3:
                    del emit_qproj.ps[(tcq, j)]
                    nc.vector.tensor_copy(QT_tiles[tcq][:, j, :], q_ps)

            emit_qproj.ps = {}

            def emit_tgt_dma(tcq):
                TGTn = tgt_pool.tile([128, KT, CHW], F16, tag="tgt", name="TGT")
                nc.sync.dma_start(out=TGTn, in_=tgtS[tcq, :, :, :])
                tgt_tiles[tcq] = TGTn

            def emit_src_dma(c):
                SRCn = stream_pool.tile(
                    [128, KT, CHW], F16, tag="stream", name="SRC"
                )
                nc.sync.dma_start(out=SRCn, in_=srcS[c, :, :, :])
                src_tiles[c] = SRCn

            OTN_tiles = {}
            o_ps_live = {}

            def emit_outproj_half(cc, ttl, dc, half, tail=False):
                OTNp = OTN_tiles[cc]
                if half == 0:
                    o_ps_live[(cc, ttl, dc)] = acc_ps.tile(
                        [128, CHW], F32, tag="acc", name="o_ps"
                    )
                o_ps = o_ps_live[(cc, ttl, dc)]
                for j in (half * 2, half * 2 + 1):
                    nc.tensor.matmul(
                        o_ps,
                        lhsT=OTNp[:, j, ttl * 128 : (ttl + 1) * 128],
                        rhs=WO[:, j, dc * CHW : (dc + 1) * CHW],
                        start=(j == 0),
                        stop=(j == PAIRS - 1),
                    )
                if half == 0:
                    return
                del o_ps_live[(cc, ttl, dc)]
                OSB = osb_pool.tile([128, CHW], F32, tag="osb", name="OSB")
                if tail:
                    nc.scalar.copy(OSB, o_ps)
                else:
                    nc.vector.tensor_copy(OSB, o_ps)
                row0 = cc * CHW + ttl * 128
                nc.sync.dma_start(
                    out=outp[row0 : row0 + 128, dc * CHW : (dc + 1) * CHW],
                    in_=OSB,
                )

            # ---- generic item queue ----
            items = []

            def emit_item(it):
                kind = it[0]
                if kind == "k":
                    emit_kproj(it[1], it[2], it[3])
                elif kind == "v":
                    emit_vproj(it[1], it[2], it[3])
                elif kind == "q":
                    emit_qproj(it[1], it[2], it[3])
                elif kind == "tgt":
                    emit_tgt_dma(it[1])
                elif kind == "src":
                    emit_src_dma(it[1])
                elif kind == "o":
                    emit_outproj_half(it[1], it[2], it[3], it[4], tail=it[5])

            def pop_items(n):
                for _ in range(n):
                    if items:
                        emit_item(items.pop(0))

            def drain_class(kinds):
                # force-emit every queued item of the given kinds (dependency
                # barrier before a burst that consumes their outputs)
                kept = []
                for it in items:
                    if it[0] in kinds:
                        emit_item(it)
                    else:
                        kept.append(it)
                items[:] = kept

            # ---- phase 0: chunk-0 projections ----
            for j in range(PAIRS):
                for part in range(4):
                    emit_kproj(0, j, part)
            for j in range(PAIRS):
                for part in range(4):
                    emit_qproj(0, j, part)
            for stl in range(4):
                for part in range(4):
                    emit_vproj(0, stl, part)

            # ---- phase A: stream source chunks 0..scn-2 ----
            # Queue the projections each round must deliver before the next
            # round (K/V of chunk r+1) and before later bursts of this round
            # (Q of t-chunks 1..3 during round 0).
            for r in range(scn - 1):
                if r == 0:
                    for tcq in (1, 2, 3):
                        for j in range(PAIRS):
                            for part in range(4):
                                items.append(("q", tcq, j, part))
                nxt = r + 1
                if nxt + 1 < scn:
                    items.append(("src", nxt + 1))
                for j in range(PAIRS):
                    for part in range(4):
                        items.append(("k", nxt, j, part))
                for stl in range(4):
                    for part in range(4):
                        items.append(("v", nxt, stl, part))

                r_sts = [s for s in range(r * 4, r * 4 + 4) if s < st_n]
                n_slots_left = [16 * len(r_sts)]
                for tcq in range(CH):
                    # dependency barrier: this t-chunk's Q must be emitted
                    drain_class({"q"}) if tcq > 0 and any(
                        it[0] == "q" and it[1] == tcq for it in items
                    ) else None
                    for j in range(PAIRS):
                        bi = tcq * PAIRS + j
                        QTc = QT_tiles[tcq]
                        av = av_ps_pool.tile(
                            [128, 2 * CHW], F32, tag="av", name="av"
                        )
                        pend = None

                        def emit_av(PT, si, st):
                            nc.tensor.matmul(
                                av[0:65, 0:CHW],
                                lhsT=VON[:, st, j * 130 : j * 130 + 65],
                                rhs=PT[:, 0:CHW],
                                start=(si == 0),
                                stop=(si == len(r_sts) - 1),
                            )
                            nc.tensor.matmul(
                                av[0:65, CHW : 2 * CHW],
                                lhsT=VON[:, st, j * 130 + 65 : j * 130 + 130],
                                rhs=PT[:, CHW : 2 * CHW],
                                start=(si == 0),
                                stop=(si == len(r_sts) - 1),
                            )

                        for si, st in enumerate(r_sts):
                            sc = sc_ps_pool.tile(
                                [128, 2 * CHW], F32, tag="sc", name="sc"
                            )
                            nc.tensor.matmul(
                                sc[:, 0:CHW],
                                lhsT=KTt[0:64, j, st * 128 : (st + 1) * 128],
                                rhs=QTc[0:64, j, :],
                                start=True,
                                stop=True,
                            )
                            nc.tensor.matmul(
                                sc[:, CHW : 2 * CHW],
                                lhsT=KTt[64:128, j, st * 128 : (st + 1) * 128],
                                rhs=QTc[64:128, j, :],
                                start=True,
                                stop=True,
                            )
                            PT = pt_pool.tile([128, 2 * CHW], F16)
                            nc.scalar.activation(PT, sc, Exp)
                            if pend is not None:
                                emit_av(*pend)
                                need = (
                                    len(items) + n_slots_left[0] - 1
                                ) // max(n_slots_left[0], 1)
                                pop_items(min(need, 3))
                            pend = (PT, si, st)
                            n_slots_left[0] -= 1
                        emit_av(*pend)
                        # fold the burst's partial sums into the fp16
                        # accumulator (copy on round 0, add afterwards)
                        if r == 0:
                            nc.vector.tensor_copy(
                                ACC[0:65, bi, :], av[0:65, :]
                            )
                        else:
                            nc.vector.tensor_add(
                                ACC[0:65, bi, :], av[0:65, :], ACC[0:65, bi, :]
                            )
                # everything the next round depends on must be emitted now
                drain_class({"k", "v", "q", "src", "tgt"})

            # ---- phase B: last source chunk, per-pair pipelined finish ----
            b_sts = list(range(bst0, st_n))
            from collections import deque

            pending_post = deque()
            for tcq in range(CH):
                if 0 < tcq:
                    for ttl in range(4):
                        for dc in range(2):
                            for half in range(2):
                                items.append(
                                    ("o", tcq - 1, ttl, dc, half, True)
                                )
                OTN = on_pool.tile([128, PAIRS, CHW], F16)
                OTN_tiles[tcq] = OTN
                for j in range(PAIRS):
                    bi = tcq * PAIRS + j
                    QTc = QT_tiles[tcq]
                    av = av_ps_pool.tile([128, 2 * CHW], F32, tag="av", name="av")
                    pend = None

                    def emit_av(PT, si, st):
                        nc.tensor.matmul(
                            av[0:65, 0:CHW],
                            lhsT=VON[:, st, j * 130 : j * 130 + 65],
                            rhs=PT[:, 0:CHW],
                            start=(si == 0),
                            stop=(si == len(b_sts) - 1),
                        )
                        nc.tensor.matmul(
                            av[0:65, CHW : 2 * CHW],
                            lhsT=VON[:, st, j * 130 + 65 : j * 130 + 130],
                            rhs=PT[:, CHW : 2 * CHW],
                            start=(si == 0),
                            stop=(si == len(b_sts) - 1),
                        )

                    for si, st in enumerate(b_sts):
                        if si == 1 and len(pending_post) >= 2:
                            pending_post.popleft()()
                        sc = sc_ps_pool.tile(
                            [128, 2 * CHW], F32, tag="sc", name="sc"
                        )
                        nc.tensor.matmul(
                            sc[:, 0:CHW],
                            lhsT=KTt[0:64, j, st * 128 : (st + 1) * 128],
                            rhs=QTc[0:64, j, :],
                            start=True,
                            stop=True,
                        )
                        nc.tensor.matmul(
                            sc[:, CHW : 2 * CHW],
                            lhsT=KTt[64:128, j, st * 128 : (st + 1) * 128],
                            rhs=QTc[64:128, j, :],
                            start=True,
                            stop=True,
                        )
                        PT = pt_pool.tile([128, 2 * CHW], F16)
                        nc.scalar.activation(PT, sc, Exp)
                        if pend is not None:
                            emit_av(*pend)
                            if tcq > 0 and j >= PAIRS - 2:
                                pop_items(2)
                        pend = (PT, si, st)
                    emit_av(*pend)

                    # total = streamed accumulator + this chunk's partials
                    AVS = avs_pool.tile([128, 2 * CHW], F32, tag="avs", name="AVS")
                    nc.vector.tensor_add(
                        AVS[0:65, :], av[0:65, :], ACC[0:65, bi, :]
                    )
                    # the custom-DVE reciprocal reads absolute partition 0:
                    # hop the denominator row there with a plain DVE copy
                    DRC = rcb_pool.tile([1, 2 * CHW], F32, tag="drc", name="DRC")
                    nc.vector.tensor_copy(DRC[0:1, :], AVS[64:65, :])
                    RC = rcb_pool.tile([1, 2 * CHW], F32, tag="rc", name="RC")
                    nc.vector.reciprocal_approx_fast(RC, DRC)
                    BCS = rcb_pool.tile([64, 2 * CHW], F32, tag="bcs", name="BCS")
                    nc.gpsimd.partition_broadcast(BCS, RC[0:1, :])

                    def _post2(AVS=AVS, BCS=BCS, OTN=OTN, j=j):
                        nc.vector.tensor_mul(
                            OTN[0:64, j, :], AVS[0:64, 0:CHW], BCS[:, 0:CHW]
                        )
                        # head B's half lands directly at partitions 64:128
                        nc.vector.tensor_mul(
                            OTN[64:128, j, :],
                            AVS[0:64, CHW : 2 * CHW],
                            BCS[:, CHW : 2 * CHW],
                        )

                    pending_post.append(_post2)
                # leftover out-projection halves of the previous t-chunk
                while items:
                    pop_items(1)

            while pending_post:
                pending_post.popleft()()
            for ttl in range(4):
                for dc in range(2):
                    for half in range(2):
                        emit_outproj_half(CH - 1, ttl, dc, half, tail=True)

    nc.finalize()
    return nc


def _get_program(st_n=None):
    global _LAST_ST_N
    if st_n is None:
        st_n = _LAST_ST_N
    if st_n not in _PROGS:
        _PROGS[st_n] = _build_program(st_n)
    _LAST_ST_N = st_n
    return _PROGS[st_n]


def kernel(src, tgt, attention_mask, Wq, Wk, Wv, Wo):
    src = np.asarray(src, dtype=np.float32)
    tgt = np.asarray(tgt, dtype=np.float32)
    mask = np.asarray(attention_mask).astype(bool)
    Wq = np.asarray(Wq, dtype=np.float32)
    Wk = np.asarray(Wk, dtype=np.float32)
    Wv = np.asarray(Wv, dtype=np.float32)
    Wo = np.asarray(Wo, dtype=np.float32)

    counts = mask.sum(axis=1)
    st_n = int(min(16, max(1, -(-int(counts.max()) // 128))))
    sp = st_n * 128
    scn = (sp + CHW - 1) // CHW
    spad = scn * CHW

    nc = _get_program(st_n)

    Wq8 = Wq * np.float32(1.0 / np.sqrt(HD))

    in_maps = []
    for core in range(N_CORES):
        b, hh = core // 2, core % 2
        rows = slice(hh * HDL, (hh + 1) * HDL)
        idx = np.nonzero(mask[b])[0]
        nb = len(idx)
        srcC = np.zeros((spad, D), dtype=np.float32)
        srcC[:nb] = src[b][idx]
        valid = np.zeros(sp, dtype=np.float16)
        valid[:nb] = 1.0
        tgtSa = np.ascontiguousarray(
            tgt[b].reshape(CH, CHW, KT, 128).transpose(0, 3, 2, 1).astype(np.float16)
        )
        srcSa = np.ascontiguousarray(
            srcC.reshape(scn, CHW, KT, 128).transpose(0, 3, 2, 1).astype(np.float16)
        )
        wqSa = np.ascontiguousarray(
            Wq8[rows].reshape(HDL, KT, 128).transpose(2, 1, 0).astype(np.float16)
        )
        wkSa = np.ascontiguousarray(
            Wk[rows].reshape(HDL, KT, 128).transpose(2, 1, 0).astype(np.float16)
        )
        wvSa = np.ascontiguousarray(
            Wv[rows].reshape(HDL, KT, 128).transpose(2, 1, 0).astype(np.float16)
        )
        woWa = np.ascontiguousarray(
            Wo[:, rows].T.reshape(PAIRS, 128, D).transpose(1, 0, 2).astype(np.float16)
        )
        validSa = np.ascontiguousarray(
            np.broadcast_to(
                valid.reshape(st_n, 128).T[:, :, None], (128, st_n, HL)
            ).astype(np.float16)
        )
        in_maps.append(
            {
                "tgtS": tgtSa,
                "srcS": srcSa,
                "wqS": wqSa,
                "wkS": wkSa,
                "wvS": wvSa,
                "woW": woWa,
                "validS": validSa,
            }
        )

    global _last_in_maps
    _last_in_maps = in_maps

    res = run_bass_kernel_spmd(nc, in_maps, list(range(N_CORES)))

    out = np.empty((B, T, D), dtype=np.float32)
    for b in range(B):
        out[b] = res.results[2 * b]["outp"] + res.results[2 * b + 1]["outp"]
    return out


# revision 31
# speedup vs baseline: 1.0321x; 1.0321x over previous
"""Cross-attention Trainium2 kernel (8 NeuronCores, SPMD), v4 "streamed".

Sharding: core = 2*b + hh  (b = batch 0..3, hh = head-half 0..1).
Each core computes attention for one batch and 8 of the 16 heads, plus the
partial output projection for its head block; the host sums the two partial
projections per batch.

v4 vs v3 (391us): the v3 structure ran all K/V projections first (~70us
with the ACT engine idle) and only then the ACT-bound attention. v4
streams over source chunks: as soon as chunk 0's K/V/Q are projected,
attention bursts start; each burst (head-pair x t-chunk) processes the
current source chunk's st-tiles and folds the partial attn@V sum into an
SBUF accumulator (fp16). Projections for later chunks ride the pop-item
slots inside the bursts. The last source chunk is processed in the v3
per-pair pipelined style, seeding the PSUM accumulation from the SBUF
accumulators, followed by softmax normalization and the out-projection.
The exp stream on ACT therefore starts at ~30us instead of ~88us.
"""

import os
import sys

import numpy as np


def _ensure_paths():
    for p in ("/opt/trn_rl_repo", "/root/.axon_site/_ro/trn_rl_repo"):
        if os.path.isdir(p) and p not in sys.path:
            sys.path.insert(0, p)


_ensure_paths()

import concourse.bass as bass  # noqa: E402
import concourse.mybir as mybir  # noqa: E402
from concourse import bacc  # noqa: E402
from concourse.bass_utils import run_bass_kernel_spmd  # noqa: E402
from concourse.tile import TileContext  # noqa: E402

B, S, T, D, H = 4, 2048, 2048, 1024, 16
HD = D // H  # 64
HL = H // 2  # heads per core: 8
HDL = HL * HD  # 512 head dims per core
PAIRS = HL // 2  # 4 head pairs (2 heads share a 128-partition tile)
KT = D // 128  # 8 contraction k-tiles for the projections
CH = 4  # t-chunks of 512
CHW = 512
F32 = mybir.dt.float32
F16 = mybir.dt.float16

N_CORES = 8

_PROGS = {}
_LAST_ST_N = None
_last_in_maps = None


def _build_program(st_n):
    sp = st_n * 128  # padded (compacted) source length
    scn = (sp + CHW - 1) // CHW  # source stream chunks of 512
    spad = scn * CHW
    bst0 = max(0, scn - 2) * 4  # first st-tile of phase B (last 2 chunks)

    nc = bacc.Bacc(None, target_bir_lowering=False, debug=False)

    tgtS = nc.dram_tensor("tgtS", [CH, 128, KT, CHW], F16, kind="ExternalInput")
    srcS = nc.dram_tensor("srcS", [scn, 128, KT, CHW], F16, kind="ExternalInput")
    wqS = nc.dram_tensor("wqS", [128, KT, HDL], F16, kind="ExternalInput")
    wkS = nc.dram_tensor("wkS", [128, KT, HDL], F16, kind="ExternalInput")
    wvS = nc.dram_tensor("wvS", [128, KT, HDL], F16, kind="ExternalInput")
    woW = nc.dram_tensor("woW", [128, PAIRS, D], F16, kind="ExternalInput")
    validS = nc.dram_tensor("validS", [128, st_n, HL], F16, kind="ExternalInput")
    outp = nc.dram_tensor("outp", [T, D], F32, kind="ExternalOutput")

    Exp = mybir.ActivationFunctionType.Exp

    with nc.allow_low_precision("fp16 matmul inputs"), TileContext(nc) as tc:
        with (
            tc.tile_pool(name="w", bufs=1) as w_pool,
            tc.tile_pool(name="kv", bufs=1) as kv_pool,
            tc.tile_pool(name="stream", bufs=2) as stream_pool,
            tc.tile_pool(name="tgt", bufs=3) as tgt_pool,
            tc.tile_pool(name="qc", bufs=4) as qc_pool,
            tc.tile_pool(name="pt", bufs=2) as pt_pool,
            tc.tile_pool(name="on", bufs=2) as on_pool,
            tc.tile_pool(name="osb", bufs=2) as osb_pool,
            tc.tile_pool(name="rcb", bufs=2) as rcb_pool,
            tc.tile_pool(name="avs", bufs=2) as avs_pool,
            tc.tile_pool(name="acc_ps", bufs=2, space="PSUM") as acc_ps,
            tc.tile_pool(name="av_ps", bufs=1, space="PSUM") as av_ps_pool,
            tc.tile_pool(name="sc_ps", bufs=2, space="PSUM") as sc_ps_pool,
        ):
            # Startup DMAs split across the two hardware DGE rings (SP =
            # nc.sync, Act = nc.scalar). The gating order: WK+SRC0 (first
            # matmul), then WQ/TGT0 (Q c0), then WV (V c0), then the rest.
            WK = w_pool.tile([128, KT, HDL], F16, tag="wk")
            nc.sync.dma_start(out=WK, in_=wkS[:, :, :])
            SRC0 = stream_pool.tile([128, KT, CHW], F16, tag="stream", name="SRC")
            nc.scalar.dma_start(out=SRC0, in_=srcS[0, :, :, :])
            WQ = w_pool.tile([128, KT, HDL], F16, tag="wq")
            nc.sync.dma_start(out=WQ, in_=wqS[:, :, :])
            TGT0 = tgt_pool.tile([128, KT, CHW], F16, tag="tgt", name="TGT")
            nc.scalar.dma_start(out=TGT0, in_=tgtS[0, :, :, :])
            WV = w_pool.tile([128, KT, HDL], F16, tag="wv")
            nc.scalar.dma_start(out=WV, in_=wvS[:, :, :])
            src_tiles = {0: SRC0}
            SRC1 = stream_pool.tile([128, KT, CHW], F16, tag="stream", name="SRC")
            nc.sync.dma_start(out=SRC1, in_=srcS[1, :, :, :])
            src_tiles[1] = SRC1
            tgt_tiles = {0: TGT0}
            TGT1 = tgt_pool.tile([128, KT, CHW], F16, tag="tgt", name="TGT")
            nc.scalar.dma_start(out=TGT1, in_=tgtS[1, :, :, :])
            tgt_tiles[1] = TGT1
            WO = w_pool.tile([128, PAIRS, D], F16, tag="wo", name="WO")
            nc.sync.dma_start(out=WO, in_=woW[:, :, :])
            VST = w_pool.tile([128, st_n, HL], F16, tag="vst", name="VST")
            nc.scalar.dma_start(out=VST, in_=validS[:, :, :])
            for tcq in (2, 3):
                TGTn = tgt_pool.tile([128, KT, CHW], F16, tag="tgt", name="TGT")
                nc.scalar.dma_start(out=TGTn, in_=tgtS[tcq, :, :, :])
                tgt_tiles[tcq] = TGTn

            # persistent K/V for the whole attention
            KTt = kv_pool.tile([128, PAIRS, spad], F16)
            VON = kv_pool.tile([128, st_n, HL * (HD + 1)], F16)
            von_heads = VON[:, :, :].rearrange("p s (h e) -> p s h e", e=HD + 1)
            nc.vector.tensor_copy(von_heads[:, :, :, HD], VST[:, :, :])
            # fp16 accumulators for the streamed partial attn@V sums: one
            # [65, 2*CHW] slab per (t-chunk, pair)
            ACC = kv_pool.tile([128, CH * PAIRS, 2 * CHW], F16, name="ACC")

            QT_tiles = {}

            # ---- granule emitters (2 matmuls each, sized to the per-st
            # slack of the ACT-bound burst loop) ----
            def emit_kproj(c, j, part):
                # K^T projection for source chunk c, pair j, k-tiles 2*part..
                if part == 0:
                    emit_kproj.ps[(c, j)] = acc_ps.tile(
                        [128, CHW], F32, tag="acc", name="k_ps"
                    )
                k_ps = emit_kproj.ps[(c, j)]
                SRC = src_tiles[c]
                for k in (2 * part, 2 * part + 1):
                    nc.tensor.matmul(
                        k_ps,
                        lhsT=WK[:, k, j * 128 : (j + 1) * 128],
                        rhs=SRC[:, k, :],
                        start=(k == 0),
                        stop=(k == KT - 1),
                    )
                if part == 3:
                    del emit_kproj.ps[(c, j)]
                    nc.vector.tensor_copy(KTt[:, j, c * CHW : (c + 1) * CHW], k_ps)

            emit_kproj.ps = {}

            def emit_vproj(c, stl, part):
                st = c * 4 + stl
                if st >= st_n:
                    return
                if part == 0:
                    emit_vproj.ps[st] = acc_ps.tile(
                        [128, CHW], F32, tag="acc", name="v_ps"
                    )
                v_ps = emit_vproj.ps[st]
                SRC = src_tiles[c]
                for k in (2 * part, 2 * part + 1):
                    nc.tensor.matmul(
                        v_ps[:, 0:HDL],
                        lhsT=SRC[:, k, stl * 128 : (stl + 1) * 128],
                        rhs=WV[:, k, :],
                        start=(k == 0),
                        stop=(k == KT - 1),
                    )
                if part == 3:
                    del emit_vproj.ps[st]
                    nc.vector.tensor_copy(
                        von_heads[:, st, :, 0:HD],
                        v_ps[:, 0:HDL].rearrange("p (h e) -> p h e", e=HD),
                    )

            emit_vproj.ps = {}

            def emit_qproj(tcq, j, part):
                if part == 0 and j == 0:
                    QT_tiles[tcq] = qc_pool.tile(
                        [128, PAIRS, CHW], F16, name="QTc"
                    )
                if part == 0:
                    emit_qproj.ps[(tcq, j)] = acc_ps.tile(
                        [128, CHW], F32, tag="acc", name="q_ps"
                    )
                q_ps = emit_qproj.ps[(tcq, j)]
                TGT = tgt_tiles[tcq]
                for k in (2 * part, 2 * part + 1):
                    nc.tensor.matmul(
                        q_ps,
                        lhsT=WQ[:, k, j * 128 : (j + 1) * 128],
                        rhs=TGT[:, k, :],
                        start=(k == 0),
                        stop=(k == KT - 1),
                    )
                if part == 3:
                    del emit_qproj.ps[(tcq, j)]
                    nc.vector.tensor_copy(QT_tiles[tcq][:, j, :], q_ps)

            emit_qproj.ps = {}

            def emit_tgt_dma(tcq):
                TGTn = tgt_pool.tile([128, KT, CHW], F16, tag="tgt", name="TGT")
                nc.sync.dma_start(out=TGTn, in_=tgtS[tcq, :, :, :])
                tgt_tiles[tcq] = TGTn

            def emit_src_dma(c):
                SRCn = stream_pool.tile(
                    [128, KT, CHW], F16, tag="stream", name="SRC"
                )
                nc.sync.dma_start(out=SRCn, in_=srcS[c, :, :, :])
                src_tiles[c] = SRCn

            OTN_tiles = {}
            o_ps_live = {}

            def emit_outproj_half(cc, ttl, dc, half, tail=False):
                OTNp = OTN_tiles[cc]
                if half == 0:
                    o_ps_live[(cc, ttl, dc)] = acc_ps.tile(
                        [128, CHW], F32, tag="acc", name="o_ps"
                    )
                o_ps = o_ps_live[(cc, ttl, dc)]
                for j in (half * 2, half * 2 + 1):
                    nc.tensor.matmul(
                        o_ps,
                        lhsT=OTNp[:, j, ttl * 128 : (ttl + 1) * 128],
                        rhs=WO[:, j, dc * CHW : (dc + 1) * CHW],
                        start=(j == 0),
                        stop=(j == PAIRS - 1),
                    )
                if half == 0:
                    return
                del o_ps_live[(cc, ttl, dc)]
                OSB = osb_pool.tile([128, CHW], F32, tag="osb", name="OSB")
                if tail:
                    nc.scalar.copy(OSB, o_ps)
                else:
                    nc.vector.tensor_copy(OSB, o_ps)
                row0 = cc * CHW + ttl * 128
                nc.sync.dma_start(
                    out=outp[row0 : row0 + 128, dc * CHW : (dc + 1) * CHW],
                    in_=OSB,
                )

            # ---- generic item queue ----
            items = []

            def emit_item(it):
                kind = it[0]
                if kind == "k":
                    emit_kproj(it[1], it[2], it[3])
                elif kind == "v":
                    emit_vproj(it[1], it[2], it[3])
                elif kind == "q":
                    emit_qproj(it[1], it[2], it[3])
                elif kind == "tgt":
                    emit_tgt_dma(it[1])
                elif kind == "src":
                    emit_src_dma(it[1])
                elif kind == "o":
                    emit_outproj_half(it[1], it[2], it[3], it[4], tail=it[5])

            def pop_items(n):
                for _ in range(n):
                    if items:
                        emit_item(items.pop(0))

            def drain_class(kinds):
                # force-emit every queued item of the given kinds (dependency
                # barrier before a burst that consumes their outputs)
                kept = []
                for it in items:
                    if it[0] in kinds:
                        emit_item(it)
                    else:
                        kept.append(it)
                items[:] = kept

            def drain_v_st(st):
                # the attn@V matmul for st must be emitted after the V
                # projection granules that write VON[st]
                kept = []
                for it in items:
                    if it[0] == "v" and it[1] * 4 + it[2] == st:
                        emit_item(it)
                    else:
                        kept.append(it)
                items[:] = kept

            def drain_q_tc(tcq):
                kept = []
                for it in items:
                    if it[0] == "q" and it[1] == tcq:
                        emit_item(it)
                    else:
                        kept.append(it)
                items[:] = kept

            # ---- phase 0: chunk-0 K and Q projections ----
            for j in range(PAIRS):
                for part in range(4):
                    emit_kproj(0, j, part)
            for j in range(PAIRS):
                for part in range(4):
                    emit_qproj(0, j, part)

            # ---- phase A: stream source chunks 0..scn-3 ----
            # Queue the projections each round must deliver: V of chunk 0 and
            # Q of t-chunks 1..3 inside round 0 (the attn@V and later bursts
            # wait on them via semaphores), K/V of chunk r+1, and in the last
            # A round also K/V of the final chunk (phase B covers two chunks).
            for r in range(max(1, scn - 2)):
                if r == 0:
                    for stl in range(4):
                        for part in range(4):
                            items.append(("v", 0, stl, part))
                    for tcq in (1, 2, 3):
                        for j in range(PAIRS):
                            for part in range(4):
                                items.append(("q", tcq, j, part))
                for nxt in range(r + 1, scn if r == max(1, scn - 2) - 1 else r + 2):
                    if nxt + 1 < scn and nxt == r + 1:
                        items.append(("src", nxt + 1))
                    for j in range(PAIRS):
                        for part in range(4):
                            items.append(("k", nxt, j, part))
                    for stl in range(4):
                        for part in range(4):
                            items.append(("v", nxt, stl, part))

                r_sts = [s for s in range(r * 4, r * 4 + 4) if s < st_n]
                n_slots_left = [16 * len(r_sts)]
                for tcq in range(CH):
                    # dependency barrier: this t-chunk's Q must be emitted
                    if tcq > 0:
                        drain_q_tc(tcq)
                    for j in range(PAIRS):
                        bi = tcq * PAIRS + j
                        QTc = QT_tiles[tcq]
                        av = av_ps_pool.tile(
                            [128, 2 * CHW], F32, tag="av", name="av"
                        )
                        pend = None

                        def emit_av(PT, si, st):
                            nc.tensor.matmul(
                                av[0:65, 0:CHW],
                                lhsT=VON[:, st, j * 130 : j * 130 + 65],
                                rhs=PT[:, 0:CHW],
                                start=(si == 0),
                                stop=(si == len(r_sts) - 1),
                            )
                            nc.tensor.matmul(
                                av[0:65, CHW : 2 * CHW],
                                lhsT=VON[:, st, j * 130 + 65 : j * 130 + 130],
                                rhs=PT[:, CHW : 2 * CHW],
                                start=(si == 0),
                                stop=(si == len(r_sts) - 1),
                            )

                        for si, st in enumerate(r_sts):
                            sc = sc_ps_pool.tile(
                                [128, 2 * CHW], F32, tag="sc", name="sc"
                            )
                            nc.tensor.matmul(
                                sc[:, 0:CHW],
                                lhsT=KTt[0:64, j, st * 128 : (st + 1) * 128],
                                rhs=QTc[0:64, j, :],
                                start=True,
                                stop=True,
                            )
                            nc.tensor.matmul(
                                sc[:, CHW : 2 * CHW],
                                lhsT=KTt[64:128, j, st * 128 : (st + 1) * 128],
                                rhs=QTc[64:128, j, :],
                                start=True,
                                stop=True,
                            )
                            PT = pt_pool.tile([128, 2 * CHW], F16)
                            nc.scalar.activation(PT, sc, Exp)
                            if pend is not None:
                                drain_v_st(pend[2])
                                emit_av(*pend)
                                need = (
                                    len(items) + n_slots_left[0] - 1
                                ) // max(n_slots_left[0], 1)
                                pop_items(min(need, 3))
                            pend = (PT, si, st)
                            n_slots_left[0] -= 1
                        drain_v_st(pend[2])
                        emit_av(*pend)
                        # fold the burst's partial sums into the fp16
                        # accumulator (copy on round 0, add afterwards)
                        if r == 0:
                            nc.vector.tensor_copy(
                                ACC[0:65, bi, :], av[0:65, :]
                            )
                        else:
                            nc.vector.tensor_add(
                                ACC[0:65, bi, :], av[0:65, :], ACC[0:65, bi, :]
                            )
                # everything the next round depends on must be emitted now
                drain_class({"k", "v", "q", "src", "tgt"})

            # ---- phase B: last source chunk, per-pair pipelined finish ----
            b_sts = list(range(bst0, st_n))
            from collections import deque

            pending_post = deque()
            for tcq in range(CH):
                if 0 < tcq:
                    for ttl in range(4):
                        for dc in range(2):
                            for half in range(2):
                                items.append(
                                    ("o", tcq - 1, ttl, dc, half, True)
                                )
                OTN = on_pool.tile([128, PAIRS, CHW], F16)
                OTN_tiles[tcq] = OTN
                for j in range(PAIRS):
                    bi = tcq * PAIRS + j
                    QTc = QT_tiles[tcq]
                    av = av_ps_pool.tile([128, 2 * CHW], F32, tag="av", name="av")
                    pend = None

                    def emit_av(PT, si, st):
                        nc.tensor.matmul(
                            av[0:65, 0:CHW],
                            lhsT=VON[:, st, j * 130 : j * 130 + 65],
                            rhs=PT[:, 0:CHW],
                            start=(si == 0),
                            stop=(si == len(b_sts) - 1),
                        )
                        nc.tensor.matmul(
                            av[0:65, CHW : 2 * CHW],
                            lhsT=VON[:, st, j * 130 + 65 : j * 130 + 130],
                            rhs=PT[:, CHW : 2 * CHW],
                            start=(si == 0),
                            stop=(si == len(b_sts) - 1),
                        )

                    for si, st in enumerate(b_sts):
                        if si == 1 and len(pending_post) >= 2:
                            pending_post.popleft()()
                        sc = sc_ps_pool.tile(
                            [128, 2 * CHW], F32, tag="sc", name="sc"
                        )
                        nc.tensor.matmul(
                            sc[:, 0:CHW],
                            lhsT=KTt[0:64, j, st * 128 : (st + 1) * 128],
                            rhs=QTc[0:64, j, :],
                            start=True,
                            stop=True,
                        )
                        nc.tensor.matmul(
                            sc[:, CHW : 2 * CHW],
                            lhsT=KTt[64:128, j, st * 128 : (st + 1) * 128],
                            rhs=QTc[64:128, j, :],
                            start=True,
                            stop=True,
                        )
                        PT = pt_pool.tile([128, 2 * CHW], F16)
                        nc.scalar.activation(PT, sc, Exp)
                        if pend is not None:
                            emit_av(*pend)
                            if tcq > 0 and j >= PAIRS - 2:
                                pop_items(2)
                        pend = (PT, si, st)
                    emit_av(*pend)

                    # total = streamed accumulator + this chunk's partials
                    AVS = avs_pool.tile([128, 2 * CHW], F32, tag="avs", name="AVS")
                    nc.vector.tensor_add(
                        AVS[0:65, :], av[0:65, :], ACC[0:65, bi, :]
                    )
                    # the custom-DVE reciprocal reads absolute partition 0:
                    # hop the denominator row there with a plain DVE copy
                    DRC = rcb_pool.tile([1, 2 * CHW], F32, tag="drc", name="DRC")
                    nc.vector.tensor_copy(DRC[0:1, :], AVS[64:65, :])
                    RC = rcb_pool.tile([1, 2 * CHW], F32, tag="rc", name="RC")
                    nc.vector.reciprocal_approx_fast(RC, DRC)
                    BCS = rcb_pool.tile([64, 2 * CHW], F32, tag="bcs", name="BCS")
                    nc.gpsimd.partition_broadcast(BCS, RC[0:1, :])

                    def _post2(AVS=AVS, BCS=BCS, OTN=OTN, j=j):
                        nc.vector.tensor_mul(
                            OTN[0:64, j, :], AVS[0:64, 0:CHW], BCS[:, 0:CHW]
                        )
                        # head B's half lands directly at partitions 64:128
                        nc.vector.tensor_mul(
                            OTN[64:128, j, :],
                            AVS[0:64, CHW : 2 * CHW],
                            BCS[:, CHW : 2 * CHW],
                        )

                    pending_post.append(_post2)
                # leftover out-projection halves of the previous t-chunk
                while items:
                    pop_items(1)

            while pending_post:
                pending_post.popleft()()
            for ttl in range(4):
                for dc in range(2):
                    for half in range(2):
                        emit_outproj_half(CH - 1, ttl, dc, half, tail=True)

    nc.finalize()
    return nc


def _get_program(st_n=None):
    global _LAST_ST_N
    if st_n is None:
        st_n = _LAST_ST_N
    if st_n not in _PROGS:
        _PROGS[st_n] = _build_program(st_n)
    _LAST_ST_N = st_n
    return _PROGS[st_n]


def kernel(src, tgt, attention_mask, Wq, Wk, Wv, Wo):
    src = np.asarray(src, dtype=np.float32)
    tgt = np.asarray(tgt, dtype=np.float32)
    mask = np.asarray(attention_mask).astype(bool)
    Wq = np.asarray(Wq, dtype=np.float32)
    Wk = np.asarray(Wk, dtype=np.float32)
    Wv = np.asarray(Wv, dtype=np.float32)
    Wo = np.asarray(Wo, dtype=np.float32)

    counts = mask.sum(axis=1)
    st_n = int(min(16, max(1, -(-int(counts.max()) // 128))))
    sp = st_n * 128
    scn = (sp + CHW - 1) // CHW
    spad = scn * CHW

    nc = _get_program(st_n)

    Wq8 = Wq * np.float32(1.0 / np.sqrt(HD))

    in_maps = []
    for core in range(N_CORES):
        b, hh = core // 2, core % 2
        rows = slice(hh * HDL, (hh + 1) * HDL)
        idx = np.nonzero(mask[b])[0]
        nb = len(idx)
        srcC = np.zeros((spad, D), dtype=np.float32)
        srcC[:nb] = src[b][idx]
        valid = np.zeros(sp, dtype=np.float16)
        valid[:nb] = 1.0
        tgtSa = np.ascontiguousarray(
            tgt[b].reshape(CH, CHW, KT, 128).transpose(0, 3, 2, 1).astype(np.float16)
        )
        srcSa = np.ascontiguousarray(
            srcC.reshape(scn, CHW, KT, 128).transpose(0, 3, 2, 1).astype(np.float16)
        )
        wqSa = np.ascontiguousarray(
            Wq8[rows].reshape(HDL, KT, 128).transpose(2, 1, 0).astype(np.float16)
        )
        wkSa = np.ascontiguousarray(
            Wk[rows].reshape(HDL, KT, 128).transpose(2, 1, 0).astype(np.float16)
        )
        wvSa = np.ascontiguousarray(
            Wv[rows].reshape(HDL, KT, 128).transpose(2, 1, 0).astype(np.float16)
        )
        woWa = np.ascontiguousarray(
            Wo[:, rows].T.reshape(PAIRS, 128, D).transpose(1, 0, 2).astype(np.float16)
        )
        validSa = np.ascontiguousarray(
            np.broadcast_to(
                valid.reshape(st_n, 128).T[:, :, None], (128, st_n, HL)
            ).astype(np.float16)
        )
        in_maps.append(
            {
                "tgtS": tgtSa,
                "srcS": srcSa,
                "wqS": wqSa,
                "wkS": wkSa,
                "wvS": wvSa,
                "woW": woWa,
                "validS": validSa,
            }
        )

    global _last_in_maps
    _last_in_maps = in_maps

    res = run_bass_kernel_spmd(nc, in_maps, list(range(N_CORES)))

    out = np.empty((B, T, D), dtype=np.float32)
    for b in range(B):
        out[b] = res.results[2 * b]["outp"] + res.results[2 * b + 1]["outp"]
    return out
